# revision 17
# baseline (speedup 1.0000x reference)
"""AttentionMIL pooling kernel for 8 Trainium2 NeuronCores.

Math (per slide b): h = tanh(X @ W1^T); s = h @ w2; a = softmax(s);
out = a^T @ X, with X [N=8192, D=1024], W1 [H=256, D], w2 [H].

Strategy (v3):
  - Data-parallel over slides: 16 slides / 8 cores = 2 per core.
  - ONE layout of X shipped (transposed, chunk-contiguous per 2048-row
    group): 93 us of HBM traffic (v1 shipped two layouts = 186 us).
  - Main matmul flipped vs v1: W1 chunks [128d, 128h] STATIONARY,
    X^T chunks [128d, 512n] moving -> h^T [h, n] PSUM. Weight loads
    (~97 ns) hide under 512-row streams (213 ns): PE ~100% efficient.
  - Scores: stationary = w2 replicated into 128 columns, so s^T lands
    PSUM-replicated on all partitions. One Exp per 2048-group gives e
    replicated in SBUF; its accum_out produces l = sum(e) for free.
  - Weighted sum (v1's 60us PE matvec tail) on DVE/Act: per d-chunk
    either a fused tensor_tensor_reduce (mult+reduce, seed-chained
    across groups into an fp32 accumulator) on DVE, or tensor_mul on
    DVE + Copy/accum_out reduce on Act for the Act-assigned chunks.
  - Softmax needs no max pass: |s| <= ||w2||_1 (~13) so exp stays in
    fp32/bf16 range; all accumulation in fp32.
"""

import sys

sys.path.insert(0, "/opt/trn_rl_repo")

import numpy as np
import ml_dtypes

import concourse.bacc as bacc
import concourse.tile as tile
from concourse import mybir
from concourse.bass_utils import run_bass_kernel_spmd

BF16 = ml_dtypes.bfloat16
B, N, D, H = 16, 8192, 1024, 256
NCORES = 8
SPC = B // NCORES          # slides per core
G = 4                      # groups per slide
GN = N // G                # 2048 rows per group
ST = 512                   # subtile rows (PE moving stream)
JPG = GN // ST             # 4 subtiles per group
KCH = D // 128             # 8 contraction chunks of D
HC = H // 128              # 2 chunks of H
GG = SPC * G               # 8 global groups per core
NST = GG * JPG             # 32 subtiles per core

# chunk -> engine assignment for the weighted-sum reduce
FUSED_CHUNKS = (0, 1, 2, 3, 4)   # DVE fused tensor_tensor_reduce
ACT_CHUNKS = (5, 6, 7)           # DVE mult + Act Copy/accum reduce

_NC_CACHE = {}


def _build_nc():
    bf = mybir.dt.bfloat16
    f32 = mybir.dt.float32
    AF = mybir.ActivationFunctionType
    OP = mybir.AluOpType

    nc = bacc.Bacc("TRN2", num_devices=NCORES)
    # xt[s, g, p, c, j] = X[s, g*GN + j, c*128 + p]
    xt = nc.declare_dram_parameter("xt", [SPC, G, 128, KCH, GN], bf, isOutput=False)
    w1t = nc.declare_dram_parameter("w1t", [128, KCH * H], bf, isOutput=False)
    w2r = nc.declare_dram_parameter("w2r", [128, HC * 128], bf, isOutput=False)
    outp = nc.declare_dram_parameter("out", [SPC, 128, KCH], f32, isOutput=True)
    outl = nc.declare_dram_parameter("outl", [SPC, 128, G], f32, isOutput=True)

    with tile.TileContext(nc) as tc:
        with tc.tile_pool(name="const", bufs=1) as constp, \
             tc.tile_pool(name="xt", bufs=3) as xtp, \
             tc.tile_pool(name="tanh", bufs=2) as tanhp, \
             tc.tile_pool(name="erep", bufs=2) as erepp, \
             tc.tile_pool(name="tmp", bufs=4) as tmpp, \
             tc.tile_pool(name="scr", bufs=2) as scrp, \
             tc.tile_pool(name="scract", bufs=2) as scractp, \
             tc.tile_pool(name="part", bufs=2) as partp, \
             tc.tile_pool(name="lpart", bufs=2) as lpartp, \
             tc.tile_pool(name="acc", bufs=2) as accp, \
             tc.tile_pool(name="hps", bufs=2, space="PSUM") as hpsp, \
             tc.tile_pool(name="sps", bufs=1, space="PSUM") as spsp:

            w1t_sb = constp.tile([128, KCH * H], bf)
            nc.gpsimd.dma_start(w1t_sb[:], w1t[:, :])
            w2r_sb = constp.tile([128, HC * 128], bf)
            nc.gpsimd.dma_start(w2r_sb[:], w2r[:, :])

            warm_sb = constp.tile([128, ST], bf)
            nc.gpsimd.memset(warm_sb[:], 0.0)
            warm_ps = hpsp.tile([128, HC * ST], f32, name="h")
            for _ in range(6):
                nc.tensor.matmul(
                    warm_ps[:, 0:ST], warm_sb[:, 0:128], warm_sb[:],
                    start=True, stop=True, skip_group_check=True,
                )

            xt_sb = {}      # gg -> sbuf tile [128, KCH*GN]
            h_tiles = {}    # st -> psum tile [128, HC*ST]
            t_tiles = {}    # st -> tanh sbuf [128, HC*ST]
            s_tiles = {}    # gg -> scores psum [128, GN]
            e_tiles = {}    # gg -> e replicated sbuf [128, GN]
            tmp_tiles = {}  # (gg, c) -> product sbuf [128, GN]
            acc_t = {}      # slide -> accumulator [128, KCH] f32
            part_t = {}     # slide -> Act-chunk partials [128, len(ACT_CHUNKS)*G]
            lpart_t = {}    # slide -> [128, G] f32

            def dma_group(gg):
                s, g = divmod(gg, G)
                t = xtp.tile([128, KCH * GN], bf)
                xt_sb[gg] = t
                # quarter loads: the PE can start after the first lands
                tv = t[:].rearrange("p (c n) -> p c n", c=KCH)
                for q in range(JPG):
                    nc.sync.dma_start(
                        tv[:, :, q * ST:(q + 1) * ST],
                        xt[s, g][:, :, q * ST:(q + 1) * ST],
                    )

            def emit_main(st):
                gg, j = divmod(st, JPG)
                x = xt_sb[gg]
                h = hpsp.tile([128, HC * ST], f32, name="h")
                h_tiles[st] = h
                for hc in range(HC):
                    for k in range(KCH):
                        nc.tensor.matmul(
                            h[:, hc * ST:(hc + 1) * ST],
                            w1t_sb[:, k * H + hc * 128: k * H + (hc + 1) * 128],
                            x[:, k * GN + j * ST: k * GN + (j + 1) * ST],
                            start=(k == 0), stop=(k == KCH - 1),
                        )

            def emit_scores(st):
                gg, j = divmod(st, JPG)
                if j == 0:
                    s_tiles[gg] = spsp.tile([128, GN], f32, name="s_ps")
                sp = s_tiles[gg]
                t = t_tiles.pop(st)
                for hc in range(HC):
                    nc.tensor.matmul(
                        sp[:, j * ST:(j + 1) * ST],
                        w2r_sb[:, hc * 128:(hc + 1) * 128],
                        t[:, hc * ST:(hc + 1) * ST],
                        start=(hc == 0), stop=(hc == HC - 1),
                    )

            def emit_tanh(st):
                h = h_tiles.pop(st)
                t = tanhp.tile([128, HC * ST], bf)
                t_tiles[st] = t
                nc.scalar.activation(t[:], h[:], AF.Tanh)

            def emit_exp(gg):
                sl, g = divmod(gg, G)
                sp = s_tiles.pop(gg)
                e = erepp.tile([128, GN], bf)
                e_tiles[gg] = e
                if sl not in lpart_t:
                    lpart_t[sl] = lpartp.tile([128, G], f32, name="lpart")
                nc.scalar.activation(
                    e[:], sp[:], AF.Exp, accum_out=lpart_t[sl][:, g:g + 1]
                )

            def emit_dve_fused(gg, c):
                sl, g = divmod(gg, G)
                if sl not in part_t:
                    part_t[sl] = partp.tile([128, KCH * G], f32, name="partials")
                x = xt_sb[gg]
                e = e_tiles[gg]
                tmp = tmpp.tile([128, GN], bf)
                nc.vector.tensor_mul(tmp[:], x[:, c * GN:(c + 1) * GN], e[:])
                scr = scrp.tile([128, GN], bf)
                nc.vector.tensor_scalar(
                    scr[:], tmp[:], 1.0, None, OP.mult, OP.add,
                    accum_out=part_t[sl][:, c * G + g: c * G + g + 1],
                )

            def emit_dve_mult(gg, c):
                x = xt_sb[gg]
                e = e_tiles[gg]
                tmp = tmpp.tile([128, GN], bf)
                tmp_tiles[(gg, c)] = tmp
                nc.vector.tensor_mul(tmp[:], x[:, c * GN:(c + 1) * GN], e[:])

            def emit_act_red(gg, c):
                sl, g = divmod(gg, G)
                if sl not in part_t:
                    part_t[sl] = partp.tile([128, KCH * G], f32, name="partials")
                tmp = tmp_tiles.pop((gg, c))
                scr = scractp.tile([128, GN], bf, name="scract")
                nc.scalar.activation(
                    scr[:], tmp[:], AF.Copy,
                    accum_out=part_t[sl][:, c * G + g: c * G + g + 1],
                )

            def emit_slide_end(sl):
                pt = part_t.pop(sl)
                acc = accp.tile([128, KCH], f32, name="acc")
                for c in range(KCH):
                    nc.vector.tensor_reduce(
                        acc[:, c:c + 1], pt[:, c * G:(c + 1) * G],
                        axis=mybir.AxisListType.X, op=OP.add,
                    )
                nc.gpsimd.dma_start(outp[sl], acc[:])
                nc.gpsimd.dma_start(outl[sl], lpart_t.pop(sl)[:])

            # DVE schedule within a group window, per subtile j:
            #   j=0: fused c0, c1   j=1: fused c2, c3
            #   j=2: fused c4, mult c5   j=3: mult c6, mult c7
            DVE_SCHED = [
                [("f", 0), ("f", 1)],
                [("f", 2), ("f", 3)],
                [("f", 4), ("m", 5)],
                [("m", 6), ("m", 7)],
            ]

            dma_group(0)
            dma_group(1)
            for vst in range(NST + 6):
                gg, j = divmod(vst, JPG)
                if j == 0 and 1 <= gg and gg + 1 < GG:
                    dma_group(gg + 1)
                if 1 <= vst <= NST:
                    emit_scores(vst - 1)
                if vst < NST:
                    emit_main(vst)
                if j == 0 and 1 <= gg <= GG:
                    emit_exp(gg - 1)
                if j == 0 and 2 <= gg <= GG + 1 and ACT_CHUNKS:
                    for c in ACT_CHUNKS[1:]:
                        emit_act_red(gg - 2, c)
                if vst < NST:
                    emit_tanh(vst)
                dg = gg - 1
                if 0 <= dg < GG:
                    for kind, c in DVE_SCHED[j]:
                        if kind == "f":
                            emit_dve_fused(dg, c)
                        else:
                            emit_dve_mult(dg, c)
                    if j == JPG - 1:
                        if ACT_CHUNKS:
                            emit_act_red(dg, ACT_CHUNKS[0])
                        xt_sb.pop(dg)
                        e_tiles.pop(dg)
                if vst >= 20 and (vst - 20) % 16 == 0:
                    emit_slide_end((vst - 20) // 16)

    nc.compile()
    return nc


def _get_nc():
    if "nc" not in _NC_CACHE:
        _NC_CACHE["nc"] = _build_nc()
    return _NC_CACHE["nc"]


def _prep_inputs(tiles_embeddings, W1, W2):
    X_bf = tiles_embeddings.astype(BF16)
    # xt[b, g, p, c, j] = X[b, g*GN + j, c*128 + p]
    xt_sw = np.ascontiguousarray(
        X_bf.reshape(B, G, GN, KCH, 128).transpose(0, 1, 4, 3, 2)
    )
    # w1t[p, k*H + h] = W1[h, k*128 + p]
    w1t = np.ascontiguousarray(
        W1.astype(BF16).reshape(H, KCH, 128).transpose(2, 1, 0)
    ).reshape(128, KCH * H)
    # w2r[p, hc*128 + m] = W2[0, hc*128 + p]  (all 128 columns identical)
    w2r2 = W2.astype(BF16).reshape(HC, 128)
    w2r = np.ascontiguousarray(
        np.broadcast_to(w2r2.T[:, :, None], (128, HC, 128))
    ).reshape(128, HC * 128)
    return [
        {
            "xt": xt_sw[c * SPC:(c + 1) * SPC],
            "w1t": w1t,
            "w2r": w2r,
        }
        for c in range(NCORES)
    ]


def _run(tiles_embeddings, W1, W2, **spmd_kwargs):
    nc = _get_nc()
    in_maps = _prep_inputs(tiles_embeddings, W1, W2)
    res = run_bass_kernel_spmd(nc, in_maps, core_ids=list(range(NCORES)), **spmd_kwargs)
    acc = np.concatenate([r["out"] for r in res.results], axis=0)    # [B, 128, KCH]
    lp = np.concatenate([r["outl"] for r in res.results], axis=0)    # [B, 128, G]
    l = lp[:, 0, :].sum(axis=1)                                      # [B]
    out = acc.transpose(0, 2, 1).reshape(B, D) / l[:, None]
    return out.astype(np.float32, copy=False), res


def kernel(tiles_embeddings, W1, W2):
    out, _ = _run(
        np.asarray(tiles_embeddings), np.asarray(W1), np.asarray(W2)
    )
    return out


# revision 18
# speedup vs baseline: 1.0281x; 1.0281x over previous
"""AttentionMIL pooling kernel for 8 Trainium2 NeuronCores.

Math (per slide b): h = tanh(X @ W1^T); s = h @ w2; a = softmax(s);
out = a^T @ X, with X [N=8192, D=1024], W1 [H=256, D], w2 [H].

Strategy (v3):
  - Data-parallel over slides: 16 slides / 8 cores = 2 per core.
  - ONE layout of X shipped (transposed, chunk-contiguous per 2048-row
    group): 93 us of HBM traffic (v1 shipped two layouts = 186 us).
  - Main matmul flipped vs v1: W1 chunks [128d, 128h] STATIONARY,
    X^T chunks [128d, 512n] moving -> h^T [h, n] PSUM. Weight loads
    (~97 ns) hide under 512-row streams (213 ns): PE ~100% efficient.
  - Scores: stationary = w2 replicated into 128 columns, so s^T lands
    PSUM-replicated on all partitions. One Exp per 2048-group gives e
    replicated in SBUF; its accum_out produces l = sum(e) for free.
  - Weighted sum (v1's 60us PE matvec tail) on DVE/Act: per d-chunk
    either a fused tensor_tensor_reduce (mult+reduce, seed-chained
    across groups into an fp32 accumulator) on DVE, or tensor_mul on
    DVE + Copy/accum_out reduce on Act for the Act-assigned chunks.
  - Softmax needs no max pass: |s| <= ||w2||_1 (~13) so exp stays in
    fp32/bf16 range; all accumulation in fp32.
"""

import sys

sys.path.insert(0, "/opt/trn_rl_repo")

import numpy as np
import ml_dtypes

import concourse.bacc as bacc
import concourse.tile as tile
from concourse import mybir
from concourse.bass_utils import run_bass_kernel_spmd

BF16 = ml_dtypes.bfloat16
B, N, D, H = 16, 8192, 1024, 256
NCORES = 8
SPC = B // NCORES          # slides per core
G = 4                      # groups per slide
GN = N // G                # 2048 rows per group
ST = 512                   # subtile rows (PE moving stream)
JPG = GN // ST             # 4 subtiles per group
KCH = D // 128             # 8 contraction chunks of D
HC = H // 128              # 2 chunks of H
GG = SPC * G               # 8 global groups per core
NST = GG * JPG             # 32 subtiles per core

# chunk -> engine assignment for the weighted-sum reduce
FUSED_CHUNKS = (0, 1, 2, 3, 4)   # DVE fused tensor_tensor_reduce
ACT_CHUNKS = (5, 6, 7)           # DVE mult + Act Copy/accum reduce

_NC_CACHE = {}


def _build_nc():
    bf = mybir.dt.bfloat16
    f32 = mybir.dt.float32
    AF = mybir.ActivationFunctionType
    OP = mybir.AluOpType

    nc = bacc.Bacc("TRN2", num_devices=NCORES)
    # xt[s, g, p, c, j] = X[s, g*GN + j, c*128 + p]
    xt = nc.declare_dram_parameter("xt", [SPC, G, 128, KCH, GN], bf, isOutput=False)
    w1t = nc.declare_dram_parameter("w1t", [128, KCH * H], bf, isOutput=False)
    w2r = nc.declare_dram_parameter("w2r", [128, HC * 128], bf, isOutput=False)
    outp = nc.declare_dram_parameter("out", [SPC, 128, KCH], f32, isOutput=True)
    outl = nc.declare_dram_parameter("outl", [SPC, 128, G], f32, isOutput=True)

    with tile.TileContext(nc) as tc:
        with tc.tile_pool(name="const", bufs=1) as constp, \
             tc.tile_pool(name="xt", bufs=3) as xtp, \
             tc.tile_pool(name="tanh", bufs=2) as tanhp, \
             tc.tile_pool(name="erep", bufs=2) as erepp, \
             tc.tile_pool(name="tmp", bufs=4) as tmpp, \
             tc.tile_pool(name="scr", bufs=2) as scrp, \
             tc.tile_pool(name="scract", bufs=2) as scractp, \
             tc.tile_pool(name="part", bufs=2) as partp, \
             tc.tile_pool(name="lpart", bufs=2) as lpartp, \
             tc.tile_pool(name="acc", bufs=2) as accp, \
             tc.tile_pool(name="hps", bufs=2, space="PSUM") as hpsp, \
             tc.tile_pool(name="sps", bufs=1, space="PSUM") as spsp:

            w1t_sb = constp.tile([128, KCH * H], bf)
            nc.gpsimd.dma_start(w1t_sb[:], w1t[:, :])
            w2r_sb = constp.tile([128, HC * 128], bf)
            nc.gpsimd.dma_start(w2r_sb[:], w2r[:, :])

            warm_sb = constp.tile([128, ST], bf)
            nc.gpsimd.memset(warm_sb[:], 0.0)
            warm_ps = hpsp.tile([128, HC * ST], f32, name="h")
            for _ in range(6):
                nc.tensor.matmul(
                    warm_ps[:, 0:ST], warm_sb[:, 0:128], warm_sb[:],
                    start=True, stop=True, skip_group_check=True,
                )

            xt_sb = {}      # gg -> sbuf tile [128, KCH*GN]
            h_tiles = {}    # st -> psum tile [128, HC*ST]
            t_tiles = {}    # st -> tanh sbuf [128, HC*ST]
            s_tiles = {}    # gg -> scores psum [128, GN]
            e_tiles = {}    # gg -> e replicated sbuf [128, GN]
            tmp_tiles = {}  # (gg, c) -> product sbuf [128, GN]
            acc_t = {}      # slide -> accumulator [128, KCH] f32
            part_t = {}     # slide -> Act-chunk partials [128, len(ACT_CHUNKS)*G]
            lpart_t = {}    # slide -> [128, G] f32

            def dma_group(gg):
                s, g = divmod(gg, G)
                t = xtp.tile([128, KCH * GN], bf)
                xt_sb[gg] = t
                # quarter loads: the PE can start after the first lands
                tv = t[:].rearrange("p (c n) -> p c n", c=KCH)
                for q in range(JPG):
                    nc.sync.dma_start(
                        tv[:, :, q * ST:(q + 1) * ST],
                        xt[s, g][:, :, q * ST:(q + 1) * ST],
                    )

            def emit_main(st):
                gg, j = divmod(st, JPG)
                x = xt_sb[gg]
                h = hpsp.tile([128, HC * ST], f32, name="h")
                h_tiles[st] = h
                for hc in range(HC):
                    for k in range(KCH):
                        nc.tensor.matmul(
                            h[:, hc * ST:(hc + 1) * ST],
                            w1t_sb[:, k * H + hc * 128: k * H + (hc + 1) * 128],
                            x[:, k * GN + j * ST: k * GN + (j + 1) * ST],
                            start=(k == 0), stop=(k == KCH - 1),
                        )

            def emit_scores(st):
                gg, j = divmod(st, JPG)
                if j == 0:
                    s_tiles[gg] = spsp.tile([128, GN], f32, name="s_ps")
                sp = s_tiles[gg]
                t = t_tiles.pop(st)
                for hc in range(HC):
                    nc.tensor.matmul(
                        sp[:, j * ST:(j + 1) * ST],
                        w2r_sb[:, hc * 128:(hc + 1) * 128],
                        t[:, hc * ST:(hc + 1) * ST],
                        start=(hc == 0), stop=(hc == HC - 1),
                    )

            def emit_tanh(st):
                h = h_tiles.pop(st)
                t = tanhp.tile([128, HC * ST], bf)
                t_tiles[st] = t
                nc.scalar.activation(t[:], h[:], AF.Tanh)

            def emit_exp(gg):
                sl, g = divmod(gg, G)
                sp = s_tiles.pop(gg)
                e = erepp.tile([128, GN], bf)
                e_tiles[gg] = e
                if sl not in lpart_t:
                    lpart_t[sl] = lpartp.tile([128, G], f32, name="lpart")
                nc.scalar.activation(
                    e[:], sp[:], AF.Exp, accum_out=lpart_t[sl][:, g:g + 1]
                )

            def emit_dve_fused(gg, c):
                sl, g = divmod(gg, G)
                if sl not in part_t:
                    part_t[sl] = partp.tile([128, KCH * G], f32, name="partials")
                x = xt_sb[gg]
                e = e_tiles[gg]
                scr = scrp.tile([128, GN], bf)
                nc.vector.scalar_tensor_tensor(
                    scr[:], x[:, c * GN:(c + 1) * GN], 1.0, e[:],
                    OP.mult, OP.mult,
                    accum_out=part_t[sl][:, c * G + g: c * G + g + 1],
                )

            def emit_dve_mult(gg, c):
                x = xt_sb[gg]
                e = e_tiles[gg]
                tmp = tmpp.tile([128, GN], bf)
                tmp_tiles[(gg, c)] = tmp
                nc.vector.tensor_mul(tmp[:], x[:, c * GN:(c + 1) * GN], e[:])

            def emit_act_red(gg, c):
                sl, g = divmod(gg, G)
                if sl not in part_t:
                    part_t[sl] = partp.tile([128, KCH * G], f32, name="partials")
                tmp = tmp_tiles.pop((gg, c))
                scr = scractp.tile([128, GN], bf, name="scract")
                nc.scalar.activation(
                    scr[:], tmp[:], AF.Copy,
                    accum_out=part_t[sl][:, c * G + g: c * G + g + 1],
                )

            def emit_slide_end(sl):
                pt = part_t.pop(sl)
                acc = accp.tile([128, KCH], f32, name="acc")
                for c in range(KCH):
                    nc.vector.tensor_reduce(
                        acc[:, c:c + 1], pt[:, c * G:(c + 1) * G],
                        axis=mybir.AxisListType.X, op=OP.add,
                    )
                nc.gpsimd.dma_start(outp[sl], acc[:])
                nc.gpsimd.dma_start(outl[sl], lpart_t.pop(sl)[:])

            # DVE schedule within a group window, per subtile j:
            #   j=0: fused c0, c1   j=1: fused c2, c3
            #   j=2: fused c4, mult c5   j=3: mult c6, mult c7
            DVE_SCHED = [
                [("f", 0), ("f", 1)],
                [("f", 2), ("f", 3)],
                [("f", 4), ("m", 5)],
                [("m", 6), ("m", 7)],
            ]

            dma_group(0)
            dma_group(1)
            for vst in range(NST + 6):
                gg, j = divmod(vst, JPG)
                if j == 0 and 1 <= gg and gg + 1 < GG:
                    dma_group(gg + 1)
                if 1 <= vst <= NST:
                    emit_scores(vst - 1)
                if vst < NST:
                    emit_main(vst)
                if j == 0 and 1 <= gg <= GG:
                    emit_exp(gg - 1)
                if j == 0 and 2 <= gg <= GG + 1 and ACT_CHUNKS:
                    for c in ACT_CHUNKS[1:]:
                        emit_act_red(gg - 2, c)
                if vst < NST:
                    emit_tanh(vst)
                dg = gg - 1
                if 0 <= dg < GG:
                    for kind, c in DVE_SCHED[j]:
                        if kind == "f":
                            emit_dve_fused(dg, c)
                        else:
                            emit_dve_mult(dg, c)
                    if j == JPG - 1:
                        if ACT_CHUNKS:
                            emit_act_red(dg, ACT_CHUNKS[0])
                        xt_sb.pop(dg)
                        e_tiles.pop(dg)
                if vst >= 20 and (vst - 20) % 16 == 0:
                    emit_slide_end((vst - 20) // 16)

    nc.compile()
    return nc


def _get_nc():
    if "nc" not in _NC_CACHE:
        _NC_CACHE["nc"] = _build_nc()
    return _NC_CACHE["nc"]


def _prep_inputs(tiles_embeddings, W1, W2):
    X_bf = tiles_embeddings.astype(BF16)
    # xt[b, g, p, c, j] = X[b, g*GN + j, c*128 + p]
    xt_sw = np.ascontiguousarray(
        X_bf.reshape(B, G, GN, KCH, 128).transpose(0, 1, 4, 3, 2)
    )
    # w1t[p, k*H + h] = W1[h, k*128 + p]
    w1t = np.ascontiguousarray(
        W1.astype(BF16).reshape(H, KCH, 128).transpose(2, 1, 0)
    ).reshape(128, KCH * H)
    # w2r[p, hc*128 + m] = W2[0, hc*128 + p]  (all 128 columns identical)
    w2r2 = W2.astype(BF16).reshape(HC, 128)
    w2r = np.ascontiguousarray(
        np.broadcast_to(w2r2.T[:, :, None], (128, HC, 128))
    ).reshape(128, HC * 128)
    return [
        {
            "xt": xt_sw[c * SPC:(c + 1) * SPC],
            "w1t": w1t,
            "w2r": w2r,
        }
        for c in range(NCORES)
    ]


def _run(tiles_embeddings, W1, W2, **spmd_kwargs):
    nc = _get_nc()
    in_maps = _prep_inputs(tiles_embeddings, W1, W2)
    res = run_bass_kernel_spmd(nc, in_maps, core_ids=list(range(NCORES)), **spmd_kwargs)
    acc = np.concatenate([r["out"] for r in res.results], axis=0)    # [B, 128, KCH]
    lp = np.concatenate([r["outl"] for r in res.results], axis=0)    # [B, 128, G]
    l = lp[:, 0, :].sum(axis=1)                                      # [B]
    out = acc.transpose(0, 2, 1).reshape(B, D) / l[:, None]
    return out.astype(np.float32, copy=False), res


def kernel(tiles_embeddings, W1, W2):
    out, _ = _run(
        np.asarray(tiles_embeddings), np.asarray(W1), np.asarray(W2)
    )
    return out
